# revision 11
# baseline (speedup 1.0000x reference)
"""Trainium2 Bass kernel for GNN message passing (edge MLP + gather + scatter-add).

  e   = lrelu(lrelu(edges @ W_e1 + b_e1) @ W_e2 + b_e2)
  out = segment_sum((nodes @ W_node)[index] * e, segmentation_index, N)

Fully dense streaming design (v2).  The previous version used gpsimd
dma_gather/dma_scatter_add, whose SWDGE descriptor generation costs ~7.5ns per
token serially on the Pool engine (~2.3ms/core for 295K tokens) and dominated
the runtime.  This version has ZERO per-token descriptor work:

 * Host-side prep (pure data layout, no arithmetic on values): edges are
   sharded by destination row (12500 rows/core), each row's edges are laid out
   contiguously, rows are grouped by degree, and the per-edge source-node rows
   nodes[index] are replicated into a dense stream in the same token order.
   Device streams are feature-major bf16 pairs: a 1024-token "block" is a
   [128, 512] tile whose partitions 0:64 hold tokens 0:512 (half A) and
   64:128 hold tokens 512:1024 (half B).
 * Device per block: three full-array matmuls
      M1: [p1A|p1B] = blockdiag(W1,W1)^T @ [edgesA;edgesB]
      M2: [mA |mB ] = blockdiag(Wn,Wn)^T @ [nodesA;nodesB]
      M3: [p2A|p2B] = blockdiag(W2,W2)^T @ [x1A;x1B]
   ACT drains psum with per-partition bias (Identity), DVE applies leaky-relu,
   Pool (gpsimd) does the message multiply msg = m * e2, and DVE
   tensor_reduce(axis=X) computes the per-destination-row segment sums
   (each 512-token half holds floor(512/d) degree-d rows contiguously, so the
   reduce is a uniform [64, K, d] -> [64, K] sum).  Results go to a resident
   SBUF staging tile; one dense DMA writes it out.  No collectives: each core
   owns a disjoint destination-row range.

The device program is compiled per input index distribution (the degree
schedule is baked in); compilation happens on first call inside kernel().
"""

import sys

for _p in ("/opt/trn_rl_repo", "/opt/pypackages"):
    if _p not in sys.path:
        sys.path.insert(0, _p)

import numpy as np
import ml_dtypes

import concourse.bacc as bacc
import concourse.mybir as mybir
import concourse.tile as tile
from concourse.bass_utils import run_bass_kernel_spmd

BF16 = ml_dtypes.bfloat16

N_NODES = 100000
NCORES = 8
NPC = N_NODES // NCORES          # 12500 destination rows per core
D = 64
HALF = 512                        # tokens per half; block = 2 halves = 1024
ALPHA = 0.01
DMA_G = 4                         # blocks per input DMA


def build_schedule(seg):
    """Degree schedule shared by all cores (input-distribution dependent).

    Returns dict with:
      halves:  list of (d, K) - each 512-token half holds K rows of degree d
      blocks:  list of (dA, KA, cA, dB, KB, cB) - per block reduce params
      colstart: per-half staging-column start
      n_blocks, cmax
      Rd: {d: R_d} common (max-over-cores) row count per degree
    """
    seg = np.asarray(seg).astype(np.int64)
    core = seg // NPC
    lr = seg - core * NPC
    maxd = 0
    cnt = {}
    for k in range(NCORES):
        deg = np.bincount(lr[core == k], minlength=NPC)
        ds, cs = np.unique(deg[deg > 0], return_counts=True)
        maxd = max(maxd, int(ds.max()) if len(ds) else 0)
        for d, c in zip(ds, cs):
            cnt.setdefault(int(d), [0] * NCORES)[k] = int(c)
    assert maxd <= HALF, f"max degree {maxd} > {HALF} unsupported"
    Rd = {d: max(v) for d, v in sorted(cnt.items())}

    # Pair same-degree halves into blocks so most blocks need only ONE
    # full-width [128, K, d] tensor_reduce (A rows on partitions 0:64 and B
    # rows on 64:128 share the same staging columns).
    halves = []          # half index -> (d, K)
    half_of_slot = {}    # d -> (base_half_index, rows_per_half)
    stash = []           # leftover odd halves (paired across degrees)
    pairs = []           # (hA, hB) half-index pairs in block order
    for d, R in Rd.items():
        rph = HALF // d
        base = len(halves)
        nh = (R + rph - 1) // rph
        for h in range(nh):
            K = rph if h < nh - 1 else R - rph * (nh - 1)
            halves.append((d, K))
        half_of_slot[d] = (base, rph)
        for i in range(nh // 2):
            pairs.append((base + 2 * i, base + 2 * i + 1))
        if nh % 2:
            stash.append(base + nh - 1)
    for i in range(0, len(stash) - 1, 2):
        pairs.append((stash[i], stash[i + 1]))
    if len(stash) % 2:
        halves.append((1, 0))     # empty pad half
        pairs.append((stash[-1], len(halves) - 1))

    colstart = [0] * len(halves)
    c = 0
    for hA, hB in pairs:
        colstart[hA] = colstart[hB] = c
        c += max(halves[hA][1], halves[hB][1])
    cmax = max(c, 1)

    blocks = []
    for hA, hB in pairs:
        dA, KA = halves[hA]
        dB, KB = halves[hB]
        blocks.append((dA, KA, colstart[hA], dB, KB, colstart[hB]))

    # half index -> stream position: pairs are in block order
    half_pos = [0] * len(halves)      # half -> (block, side) flat half slot
    for b, (hA, hB) in enumerate(pairs):
        half_pos[hA] = 2 * b
        half_pos[hB] = 2 * b + 1

    return dict(halves=halves, blocks=blocks, colstart=colstart,
                n_blocks=len(blocks), cmax=cmax, Rd=Rd,
                half_of_slot=half_of_slot, half_pos=half_pos)


def build_kernel(sched):
    nb = sched["n_blocks"]
    cmax = sched["cmax"]
    scols = nb * HALF

    nc = bacc.Bacc("TRN2", target_bir_lowering=False)

    edges_fm = nc.dram_tensor("edges_fm", [128, scols], mybir.dt.bfloat16,
                              kind="ExternalInput")
    nodesg_fm = nc.dram_tensor("nodesg_fm", [128, scols], mybir.dt.bfloat16,
                               kind="ExternalInput")
    w1_d = nc.dram_tensor("w1d", [128, 128], mybir.dt.bfloat16,
                          kind="ExternalInput")
    wn_d = nc.dram_tensor("wnd", [128, 128], mybir.dt.bfloat16,
                          kind="ExternalInput")
    w2_d = nc.dram_tensor("w2d", [128, 128], mybir.dt.bfloat16,
                          kind="ExternalInput")
    b1_d = nc.dram_tensor("b1d", [128, 1], mybir.dt.float32,
                          kind="ExternalInput")
    b2_d = nc.dram_tensor("b2d", [128, 1], mybir.dt.float32,
                          kind="ExternalInput")
    out_d = nc.dram_tensor("out", [128, cmax], mybir.dt.float32,
                           kind="ExternalOutput")

    gw = HALF * DMA_G

    with tile.TileContext(nc) as tc:
        with tc.tile_pool(name="const", bufs=1) as cpool, \
             tc.tile_pool(name="stage", bufs=1) as spool, \
             tc.tile_pool(name="ein", bufs=2) as epool, \
             tc.tile_pool(name="nin", bufs=2) as npool, \
             tc.tile_pool(name="x1", bufs=3) as x1pool, \
             tc.tile_pool(name="e2", bufs=3) as e2pool, \
             tc.tile_pool(name="msg", bufs=3) as mpool, \
             tc.tile_pool(name="ps1", bufs=3, space="PSUM") as ps1, \
             tc.tile_pool(name="ps2", bufs=2, space="PSUM") as ps2, \
             tc.tile_pool(name="ps3", bufs=3, space="PSUM") as ps3:

            w1 = cpool.tile([128, 128], mybir.dt.bfloat16, tag="w1")
            wn = cpool.tile([128, 128], mybir.dt.bfloat16, tag="wn")
            w2 = cpool.tile([128, 128], mybir.dt.bfloat16, tag="w2")
            b1 = cpool.tile([128, 1], mybir.dt.float32, tag="b1")
            b2 = cpool.tile([128, 1], mybir.dt.float32, tag="b2")
            nc.sync.dma_start(out=w1[:], in_=w1_d[:])
            nc.sync.dma_start(out=wn[:], in_=wn_d[:])
            nc.sync.dma_start(out=w2[:], in_=w2_d[:])
            nc.sync.dma_start(out=b1[:], in_=b1_d[:])
            nc.sync.dma_start(out=b2[:], in_=b2_d[:])

            stage = spool.tile([128, cmax], mybir.dt.float32, tag="stage")

            etiles = {}
            ntiles = {}
            for b in range(nb):
                g, off = divmod(b, DMA_G)
                if off == 0:
                    w = min(gw, (nb - g * DMA_G) * HALF)
                    et = epool.tile([128, w], mybir.dt.bfloat16, tag="et")
                    nt = npool.tile([128, w], mybir.dt.bfloat16, tag="nt")
                    nc.sync.dma_start(
                        out=et[:], in_=edges_fm[:, g * gw:g * gw + w])
                    nc.gpsimd.dma_start(
                        out=nt[:], in_=nodesg_fm[:, g * gw:g * gw + w])
                    etiles[g], ntiles[g] = et, nt

                er = etiles[g][:, off * HALF:(off + 1) * HALF]
                nr = ntiles[g][:, off * HALF:(off + 1) * HALF]

                p1 = ps1.tile([128, HALF], mybir.dt.float32, tag="p1")
                nc.tensor.matmul(p1[:], w1[:], er, start=True, stop=True)
                pm = ps2.tile([128, HALF], mybir.dt.float32, tag="pm")
                nc.tensor.matmul(pm[:], wn[:], nr, start=True, stop=True)

                x1 = x1pool.tile([128, HALF], mybir.dt.bfloat16, tag="x1")
                nc.scalar.activation(x1[:], p1[:],
                                     mybir.ActivationFunctionType.Lrelu,
                                     bias=b1[:, :1], scale=1.0, alpha=ALPHA)

                p2 = ps3.tile([128, HALF], mybir.dt.float32, tag="p2")
                nc.tensor.matmul(p2[:], w2[:], x1[:], start=True, stop=True)

                e2 = e2pool.tile([128, HALF], mybir.dt.bfloat16, tag="e2")
                nc.scalar.activation(e2[:], p2[:],
                                     mybir.ActivationFunctionType.Lrelu,
                                     bias=b2[:, :1], scale=1.0, alpha=ALPHA)

                msg = mpool.tile([128, HALF], mybir.dt.bfloat16, tag="msg")
                nc.vector.tensor_tensor(out=msg[:], in0=pm[:], in1=e2[:],
                                        op=mybir.AluOpType.mult)

                dA, KA, cA, dB, KB, cB = sched["blocks"][b]
                if dA == dB and KA == KB and KA:
                    nc.vector.tensor_reduce(
                        out=stage[:, cA:cA + KA],
                        in_=msg[:, 0:KA * dA].rearrange(
                            "p (r d) -> p r d", d=dA),
                        axis=mybir.AxisListType.X, op=mybir.AluOpType.add)
                else:
                    if KA:
                        nc.vector.tensor_reduce(
                            out=stage[0:64, cA:cA + KA],
                            in_=msg[0:64, 0:KA * dA].rearrange(
                                "p (r d) -> p r d", d=dA),
                            axis=mybir.AxisListType.X, op=mybir.AluOpType.add)
                    if KB:
                        nc.vector.tensor_reduce(
                            out=stage[64:128, cB:cB + KB],
                            in_=msg[64:128, 0:KB * dB].rearrange(
                                "p (r d) -> p r d", d=dB),
                            axis=mybir.AxisListType.X, op=mybir.AluOpType.add)

            nc.sync.dma_start(out=out_d[:], in_=stage[:])

    nc.compile()
    return nc


def host_prep(sched, nodes, edges, seg, index, W_node, W_e1, b_e1, W_e2, b_e2):
    """Build per-core dense streams + output row maps.  Layout only."""
    seg = np.asarray(seg).astype(np.int64)
    index = np.asarray(index).astype(np.int64)
    nodes = np.asarray(nodes, dtype=np.float32)
    edges = np.asarray(edges, dtype=np.float32)

    nb = sched["n_blocks"]
    scols = nb * HALF
    ntok = nb * 2 * HALF

    def dupblock(w):
        z = np.zeros((128, 128), np.float32)
        z[0:64, 0:64] = w
        z[64:128, 64:128] = w
        return np.ascontiguousarray(z.astype(BF16))

    w1d = dupblock(np.asarray(W_e1, np.float32))
    wnd = dupblock(np.asarray(W_node, np.float32))
    w2d = dupblock(np.asarray(W_e2, np.float32))
    b1d = np.ascontiguousarray(
        np.tile(np.asarray(b_e1, np.float32), 2)[:, None])
    b2d = np.ascontiguousarray(
        np.tile(np.asarray(b_e2, np.float32), 2)[:, None])

    core = seg // NPC
    lr_all = seg - core * NPC

    in_maps = []
    row_maps = []
    for k in range(NCORES):
        eids = np.flatnonzero(core == k)
        lr = lr_all[eids]
        deg = np.bincount(lr, minlength=NPC)

        # sort edges by (row degree, row id) so each row's edges are
        # contiguous and rows are grouped by degree
        dkey = deg[lr].astype(np.int64)
        order = np.argsort(dkey * 16384 + lr, kind="stable")
        eids = eids[order]
        lr_s = lr[order]

        # per-edge occurrence rank within its row
        newrow = np.ones(len(lr_s), dtype=bool)
        newrow[1:] = lr_s[1:] != lr_s[:-1]
        rstart = np.maximum.accumulate(
            np.where(newrow, np.arange(len(lr_s)), 0))
        occ = np.arange(len(lr_s)) - rstart

        # slot (rank of row within its degree group) per row
        rows_first = np.flatnonzero(newrow)          # first edge idx per row
        row_ids = lr_s[rows_first]                   # row per group
        row_degs = deg[row_ids]
        # rows are already sorted by (deg, row); slot = rank within degree
        slot = np.zeros(len(row_ids), np.int64)
        for d in np.unique(row_degs):
            m = row_degs == d
            slot[m] = np.arange(m.sum())
        # token start position per row: p0 = (base_h + slot//rph)*512
        #                                + (slot%rph)*d
        base = np.zeros(len(row_ids), np.int64)
        rphv = np.zeros(len(row_ids), np.int64)
        for d in np.unique(row_degs):
            bh, rph = sched["half_of_slot"][int(d)]
            m = row_degs == d
            base[m] = bh
            rphv[m] = rph
        halfidx = base + slot // rphv
        half_pos = np.asarray(sched["half_pos"], np.int64)
        p0 = half_pos[halfidx] * HALF + (slot % rphv) * row_degs
        # per edge: p = p0[rowgrp] + occ
        rowgrp = np.cumsum(newrow) - 1
        p = p0[rowgrp] + occ
        assert p.max() < ntok

        ef = np.zeros((ntok, D), np.float32)
        ng = np.zeros((ntok, D), np.float32)
        ef[p] = edges[eids]
        ng[p] = nodes[index[eids]]
        # [ntok, 64] -> [128, scols]: block b, side s, j, feat
        ef4 = ef.reshape(nb, 2, HALF, D).transpose(1, 3, 0, 2)
        ng4 = ng.reshape(nb, 2, HALF, D).transpose(1, 3, 0, 2)
        in_maps.append({
            "edges_fm": np.ascontiguousarray(
                ef4.reshape(128, scols).astype(BF16)),
            "nodesg_fm": np.ascontiguousarray(
                ng4.reshape(128, scols).astype(BF16)),
            "w1d": w1d, "wnd": wnd, "w2d": w2d, "b1d": b1d, "b2d": b2d,
        })
        # output map: local row -> (side, stage column)
        colstart = np.asarray(sched["colstart"], np.int64)
        rm_side = half_pos[halfidx] % 2
        rm_col = colstart[halfidx] + slot % rphv
        row_maps.append((row_ids, rm_side, rm_col))
    return in_maps, row_maps


_NC_CACHE = {}


def _get_nc(sched):
    key = tuple(sched["halves"])
    if key not in _NC_CACHE:
        _NC_CACHE[key] = build_kernel(sched)
    return _NC_CACHE[key]


def kernel(nodes, edges, segmentation_index, index, W_node, W_e1, b_e1, W_e2,
           b_e2, _trace=False):
    sched = build_schedule(segmentation_index)
    nc = _get_nc(sched)
    in_maps, row_maps = host_prep(sched, nodes, edges, segmentation_index,
                                  index, W_node, W_e1, b_e1, W_e2, b_e2)
    res = run_bass_kernel_spmd(nc, in_maps, core_ids=list(range(NCORES)),
                               trace=_trace)
    out = np.zeros((N_NODES, D), np.float32)
    for k in range(NCORES):
        st = np.asarray(res.results[k]["out"], np.float32)
        row_ids, rm_side, rm_col = row_maps[k]
        vals = np.where(rm_side[:, None] == 0,
                        st[0:64, rm_col].T, st[64:128, rm_col].T)
        out[k * NPC + row_ids] = vals
    if _trace:
        return out, res
    return out
